# revision 11
# baseline (speedup 1.0000x reference)
"""Distributed multi-head attention kernel for 8 TRN2 NeuronCores.

Problem: x[4, 2048, 1024] @ w_qkv[1024, 3072] -> qkv -> 16-head attention
         -> out[4, 2048, 1024], fp32.

Sharding (data parallel batch x tensor parallel heads):
  core c handles batch b = c // 2 and heads h0 = (c % 2) * 8 .. h0 + 8.
  Each core receives x_b [2048, 1024] and the w_qkv column slice for its
  8 heads ([1024, 1536] = q|k|v each 512 cols), produces out[b, :, 512-slice].
  All 64 (batch, head) attention problems are independent -> no collectives.

Per-core kernel (all matmuls bf16 with fp32 PSUM accumulation):
  1. x -> bf16 -> PE-transpose -> xT [c, i]       (contraction dim on partitions)
  2. qkT = w_qk.T @ x.T (per f-tile), v = x @ w_v (natural layout)
     v is stored with a fused ones-column per head: v'[:, h] = [v_h | 1]
  3. per head-pair (heads 2hp/2hp+1 share an f-tile at partition bases 0/64):
     dotsT for both heads row-packed into one [128, 1024] psum (column
     halves, concurrent matmuls); one Exp ACTIVATE covers both heads;
     U'[d|Z, i] += v'_h.T @ P (PV matmul, one j-step behind exp; row 64 =
     softmax denominator Z via the ones column)
  4. PE-transpose U' -> [i, d|Z]; out = U * (1/Z) (DVE); DMA out.

Emission order is pipeline-aware: w first, then x-tile batches interleaved
with pair-0 QKV groups (PE executes in order; first dots only needs w +
4 x-tiles), then attention pairs with the next pair's QKV emitted between.
"""

import numpy as np

B, N, DIM = 4, 2048, 1024
HEADS, DIM_HEAD = 16, 64
INNER = HEADS * DIM_HEAD
HPC = 8                 # heads per core
FQ = HPC * DIM_HEAD     # 512 = per-core q/k/v column count
NCORES = 8

P = 128
CT = DIM // P           # 8 c-tiles (contraction)
IT = N // P             # 16 i-tiles
JT = N // P             # 16 j-tiles

_CACHE = {}


def _build():
    import concourse.bass as bass
    import concourse.mybir as mybir
    import concourse.tile as tile
    from concourse import bacc
    from concourse.masks import make_identity

    f32 = mybir.dt.float32
    bf16 = mybir.dt.bfloat16
    Exp = mybir.ActivationFunctionType.Exp
    ds = bass.ds

    nc = bacc.Bacc(None, target_bir_lowering=False)
    x_d = nc.dram_tensor("x", [N, DIM], f32, kind="ExternalInput")
    w_d = nc.dram_tensor("w", [DIM, 3 * FQ], f32, kind="ExternalInput")
    o_d = nc.dram_tensor("o", [N, FQ], f32, kind="ExternalOutput")

    with tile.TileContext(nc) as tc, \
         tc.tile_pool(name="persist", bufs=1) as persist, \
         tc.tile_pool(name="wload", bufs=2) as wload, \
         tc.tile_pool(name="xload", bufs=3) as xload, \
         tc.tile_pool(name="xcast", bufs=3) as xcast, \
         tc.tile_pool(name="scratchp", bufs=2, space="PSUM") as scratchp, \
         tc.tile_pool(name="dotsp", bufs=2, space="PSUM") as dotsp, \
         tc.tile_pool(name="upp", bufs=1, space="PSUM") as upp, \
         tc.tile_pool(name="ptp", bufs=3) as ptp, \
         tc.tile_pool(name="uep", bufs=2) as uep, \
         tc.tile_pool(name="recp", bufs=2) as recp, \
         tc.tile_pool(name="normp", bufs=4) as normp:

        ident = persist.tile([P, P], bf16, tag="ident", name="ident")
        make_identity(nc, ident[:])

        xT = persist.tile([P, CT, N], bf16, tag="xT", name="xT")
        wsb = persist.tile([P, CT, 3 * FQ], bf16, tag="wsb", name="wsb")
        qkT = persist.tile([P, CT, N], bf16, tag="qkT", name="qkT")
        vp = persist.tile([P, JT, HPC * 65], bf16, tag="vp", name="vp")

        vp_heads = vp[:].rearrange("p j (h c) -> p j h c", c=65)
        nc.vector.memset(vp_heads[:, :, :, 64:65], 1.0)

        def emit_w():
            for ct in range(CT):
                wf = wload.tile([P, 3 * FQ], f32, tag="wf")
                nc.sync.dma_start(wf[:], w_d[ds(ct * P, P), :])
                nc.vector.tensor_copy(wsb[:, ct], wf[:])

        def emit_x_tile(it):
            xf = xload.tile([P, DIM], f32, tag="xf")
            nc.sync.dma_start(xf[:], x_d[ds(it * P, P), :])
            xb = xcast.tile([P, DIM], bf16, tag="xb")
            nc.vector.tensor_copy(xb[:], xf[:])
            for ct in range(CT):
                tp = scratchp.tile([P, P], bf16, tag="sc")
                nc.tensor.transpose(tp[:], xb[:, ds(ct * P, P)], ident[:])
                nc.vector.tensor_copy(xT[:, ct, ds(it * P, P)], tp[:])

        def emit_qk_group(ft, ic):
            ps = scratchp.tile([P, 512], f32, tag="sc")
            for ct in range(CT):
                nc.tensor.matmul(
                    ps[:],
                    wsb[:, ct, ds(ft * P, P)],
                    xT[:, ct, ds(ic * 512, 512)],
                    start=(ct == 0), stop=(ct == CT - 1),
                )
            nc.vector.tensor_copy(qkT[:, ft, ds(ic * 512, 512)], ps[:])

        def emit_v_group(hp, it):
            ps = scratchp.tile([P, P], f32, tag="sc")
            for ct in range(CT):
                nc.tensor.matmul(
                    ps[:],
                    xT[:, ct, ds(it * P, P)],
                    wsb[:, ct, ds(2 * FQ + hp * P, P)],
                    start=(ct == 0), stop=(ct == CT - 1),
                )
            nc.vector.tensor_copy(
                vp_heads[:, it, ds(2 * hp, 2), 0:64],
                ps[:].rearrange("p (h c) -> p h c", c=64),
            )

        def emit_qkv_for_pair(hp):
            for ic in range(4):
                emit_qk_group(hp, ic)
                emit_qk_group(4 + hp, ic)
            for it in range(IT):
                emit_v_group(hp, it)

        def emit_epilogue(ups, ic, h):
            # U'[d|Z, 512] -> transpose 128-blocks -> normalize -> DMA out
            ue = uep.tile([65, 512], bf16, tag="ue")
            nc.vector.tensor_copy(ue[:], ups[:])
            for b in range(4):
                tp = scratchp.tile([P, 65], bf16, tag="sc")
                nc.tensor.transpose(
                    tp[:], ue[:, ds(b * P, P)], ident[0:65, 0:65]
                )
                rec = recp.tile([P, 1], f32, tag="rec")
                nc.vector.reciprocal(rec[:], tp[:, 64:65])
                nrm = normp.tile([P, 64], f32, tag="nrm")
                nc.vector.tensor_scalar_mul(nrm[:], tp[:, 0:64], rec[:])
                it = ic * 4 + b
                nc.sync.dma_start(
                    o_d[ds(it * P, P), ds(h * 64, 64)], nrm[:]
                )

        def emit_attention_pair(hp):
            # heads hA = 2hp (partitions 0:64), hB = 2hp+1 (64:128) share
            # f-tiles qft/kft; dots for both packed into one [128, 1024]
            # psum (column halves) -> one exp ACTIVATE covers both.
            hA, hB = 2 * hp, 2 * hp + 1
            qft, kft = hp, 4 + hp
            for ic in range(4):          # i-chunks of 512
                upsA = upp.tile([65, 512], f32, tag="upsA")
                upsB = upp.tile([65, 512], f32, tag="upsB")
                prev_pt = None
                for j in range(JT):
                    dt_ = dotsp.tile([P, 1024], f32, tag="dt")
                    nc.tensor.matmul(
                        dt_[:, 0:512],
                        qkT[0:64, kft, ds(j * P, P)],
                        qkT[0:64, qft, ds(ic * 512, 512)],
                        start=True, stop=True,
                    )
                    nc.tensor.matmul(
                        dt_[:, 512:1024],
                        qkT[64:128, kft, ds(j * P, P)],
                        qkT[64:128, qft, ds(ic * 512, 512)],
                        start=True, stop=True,
                    )
                    pt = ptp.tile([P, 1024], bf16, tag="pt")
                    nc.scalar.activation(pt[:], dt_[:], Exp, scale=0.125)
                    if prev_pt is not None:
                        jj = j - 1
                        nc.tensor.matmul(
                            upsA[:], vp_heads[:, jj, hA, :],
                            prev_pt[:, 0:512],
                            start=(jj == 0), stop=False,
                        )
                        nc.tensor.matmul(
                            upsB[:], vp_heads[:, jj, hB, :],
                            prev_pt[:, 512:1024],
                            start=(jj == 0), stop=False,
                        )
                    prev_pt = pt
                jj = JT - 1
                nc.tensor.matmul(
                    upsA[:], vp_heads[:, jj, hA, :], prev_pt[:, 0:512],
                    start=False, stop=True,
                )
                nc.tensor.matmul(
                    upsB[:], vp_heads[:, jj, hB, :], prev_pt[:, 512:1024],
                    start=False, stop=True,
                )
                emit_epilogue(upsA, ic, hA)
                emit_epilogue(upsB, ic, hB)

        # ---- emission ----
        emit_w()
        # x-tile batches interleaved with pair-0 QKV so the in-order PE
        # queue reaches the first dots as soon as w + 4 x-tiles are in.
        for batch in range(4):
            for it in range(4 * batch, 4 * batch + 4):
                emit_x_tile(it)
            emit_qk_group(0, batch)      # pair-0 q chunk
            emit_qk_group(4, batch)      # pair-0 k chunk
            for it in range(4 * batch, 4 * batch + 4):
                emit_v_group(0, it)
        emit_attention_pair(0)
        for hp in range(1, 4):
            emit_qkv_for_pair(hp)
            emit_attention_pair(hp)

    nc.finalize()
    return nc


def _get_nc():
    if "nc" not in _CACHE:
        _CACHE["nc"] = _build()
    return _CACHE["nc"]


def kernel(x: np.ndarray, w_qkv: np.ndarray) -> np.ndarray:
    from concourse.bass_utils import run_bass_kernel_spmd

    x = np.asarray(x, dtype=np.float32)
    w_qkv = np.asarray(w_qkv, dtype=np.float32)

    in_maps = []
    for c in range(NCORES):
        b, hh = c // 2, c % 2
        qo = hh * FQ
        ws = np.concatenate(
            [w_qkv[:, qo:qo + FQ],
             w_qkv[:, INNER + qo:INNER + qo + FQ],
             w_qkv[:, 2 * INNER + qo:2 * INNER + qo + FQ]], axis=1)
        in_maps.append({
            "x": np.ascontiguousarray(x[b]),
            "w": np.ascontiguousarray(ws),
        })

    nc = _get_nc()
    res = run_bass_kernel_spmd(nc, in_maps, core_ids=list(range(NCORES)))

    out = np.empty((B, N, INNER), np.float32)
    for c in range(NCORES):
        b, hh = c // 2, c % 2
        out[b, :, hh * FQ:(hh + 1) * FQ] = res.results[c]["o"]
    return out


# revision 14
# speedup vs baseline: 1.1164x; 1.1164x over previous
"""Distributed multi-head attention kernel for 8 TRN2 NeuronCores.

Problem: x[4, 2048, 1024] @ w_qkv[1024, 3072] -> qkv -> 16-head attention
         -> out[4, 2048, 1024], fp32.

Sharding (data parallel batch x tensor parallel heads):
  core c handles batch b = c // 2 and heads h0 = (c % 2) * 8 .. h0 + 8.
  Each core receives x_b [2048, 1024] and the w_qkv column slice for its
  8 heads ([1024, 1536] = q|k|v each 512 cols), produces out[b, :, 512-slice].
  All 64 (batch, head) attention problems are independent -> no collectives.

Per-core kernel (all matmuls bf16 with fp32 PSUM accumulation):
  1. x -> bf16 -> PE-transpose -> xT [c, i]       (contraction dim on partitions)
  2. qkT = w_qk.T @ x.T (per f-tile), v = x @ w_v (natural layout)
     v is stored with a fused ones-column per head: v'[:, h] = [v_h | 1]
  3. per head-pair (heads 2hp/2hp+1 share an f-tile at partition bases 0/64):
     dotsT for both heads row-packed into one [128, 1024] psum (column
     halves, concurrent matmuls); one Exp ACTIVATE covers both heads;
     U'[d|Z, i] += v'_h.T @ P (PV matmul, one j-step behind exp; row 64 =
     softmax denominator Z via the ones column)
  4. PE-transpose U' -> [i, d|Z]; out = U * (1/Z) (DVE); DMA out.

Emission order is pipeline-aware: w first, then x-tile batches interleaved
with pair-0 QKV groups (PE executes in order; first dots only needs w +
4 x-tiles), then attention pairs with the next pair's QKV emitted between.
"""

import numpy as np

B, N, DIM = 4, 2048, 1024
HEADS, DIM_HEAD = 16, 64
INNER = HEADS * DIM_HEAD
HPC = 8                 # heads per core
FQ = HPC * DIM_HEAD     # 512 = per-core q/k/v column count
NCORES = 8

P = 128
CT = DIM // P           # 8 c-tiles (contraction)
IT = N // P             # 16 i-tiles
JT = N // P             # 16 j-tiles

_CACHE = {}


def _build():
    import concourse.bass as bass
    import concourse.mybir as mybir
    import concourse.tile as tile
    from concourse import bacc
    from concourse.masks import make_identity

    f32 = mybir.dt.float32
    bf16 = mybir.dt.bfloat16
    Exp = mybir.ActivationFunctionType.Exp
    ds = bass.ds

    nc = bacc.Bacc(None, target_bir_lowering=False)
    x_d = nc.dram_tensor("x", [N, DIM], f32, kind="ExternalInput")
    w_d = nc.dram_tensor("w", [DIM, 3 * FQ], f32, kind="ExternalInput")
    o_d = nc.dram_tensor("o", [N, FQ], f32, kind="ExternalOutput")

    with tile.TileContext(nc) as tc, \
         tc.tile_pool(name="persist", bufs=1) as persist, \
         tc.tile_pool(name="wload", bufs=2) as wload, \
         tc.tile_pool(name="xload", bufs=3) as xload, \
         tc.tile_pool(name="xcast", bufs=3) as xcast, \
         tc.tile_pool(name="qkvp", bufs=1, space="PSUM") as qkvp, \
         tc.tile_pool(name="ptp", bufs=3) as ptp, \
         tc.tile_pool(name="uep", bufs=2) as uep, \
         tc.tile_pool(name="recp", bufs=2) as recp, \
         tc.tile_pool(name="normp", bufs=4) as normp:

        ident = persist.tile([P, P], bf16, tag="ident", name="ident")
        make_identity(nc, ident[:])

        xT = persist.tile([P, CT, N], bf16, tag="xT", name="xT")
        wsb = persist.tile([P, CT, 3 * FQ], bf16, tag="wsb", name="wsb")
        qkT = persist.tile([P, CT, N], bf16, tag="qkT", name="qkT")
        vp = persist.tile([P, JT, HPC * 65], bf16, tag="vp", name="vp")

        vp_heads = vp[:].rearrange("p j (h c) -> p j h c", c=65)
        nc.vector.memset(vp_heads[:, :, :, 64:65], 1.0)

        def emit_w():
            for ct in range(CT):
                wf = wload.tile([P, 3 * FQ], f32, tag="wf")
                nc.sync.dma_start(wf[:], w_d[ds(ct * P, P), :])
                nc.vector.tensor_copy(wsb[:, ct], wf[:])

        def emit_x_tile(it, tpsx):
            xf = xload.tile([P, DIM], f32, tag="xf")
            nc.sync.dma_start(xf[:], x_d[ds(it * P, P), :])
            xb = xcast.tile([P, DIM], bf16, tag="xb")
            nc.vector.tensor_copy(xb[:], xf[:])
            for ct in range(CT):
                tp = tpsx.tile([P, P], bf16, tag="tpsx")
                nc.tensor.transpose(tp[:], xb[:, ds(ct * P, P)], ident[:])
                nc.vector.tensor_copy(xT[:, ct, ds(it * P, P)], tp[:])

        def emit_qk_group(ft, ic):
            ps = qkvp.tile([P, 512], f32, tag="qkv")
            for ct in range(CT):
                nc.tensor.matmul(
                    ps[:],
                    wsb[:, ct, ds(ft * P, P)],
                    xT[:, ct, ds(ic * 512, 512)],
                    start=(ct == 0), stop=(ct == CT - 1),
                )
            nc.vector.tensor_copy(qkT[:, ft, ds(ic * 512, 512)], ps[:])

        def emit_v_group(hp, it):
            ps = qkvp.tile([P, P], f32, tag="qkv")
            for ct in range(CT):
                nc.tensor.matmul(
                    ps[:],
                    xT[:, ct, ds(it * P, P)],
                    wsb[:, ct, ds(2 * FQ + hp * P, P)],
                    start=(ct == 0), stop=(ct == CT - 1),
                )
            nc.vector.tensor_copy(
                vp_heads[:, it, ds(2 * hp, 2), 0:64],
                ps[:].rearrange("p (h c) -> p h c", c=64),
            )

        def emit_qkv_for_pair(hp):
            for ic in range(4):
                emit_qk_group(hp, ic)
                emit_qk_group(4 + hp, ic)
            for it in range(IT):
                emit_v_group(hp, it)

        def emit_epilogue(ups, ic, h, tpp):
            # U'[d|Z, 512] -> transpose 128-blocks -> normalize -> DMA out
            ue = uep.tile([65, 512], bf16, tag="ue")
            nc.vector.tensor_copy(ue[:], ups[:])
            for b in range(4):
                tp = tpp.tile([P, 65], bf16, tag="tp")
                nc.tensor.transpose(
                    tp[:], ue[:, ds(b * P, P)], ident[0:65, 0:65]
                )
                rec = recp.tile([P, 1], f32, tag="rec")
                nc.vector.reciprocal(rec[:], tp[:, 64:65])
                nrm = normp.tile([P, 64], f32, tag="nrm")
                nc.vector.tensor_scalar_mul(nrm[:], tp[:, 0:64], rec[:])
                it = ic * 4 + b
                nc.sync.dma_start(
                    o_d[ds(it * P, P), ds(h * 64, 64)], nrm[:]
                )

        def emit_attention_pair(hp, dotsp, upp, tpp):
            # heads hA = 2hp (partitions 0:64), hB = 2hp+1 (64:128) share
            # f-tiles qft/kft; dots for both packed into one [128, 1024]
            # psum (column halves) -> one exp ACTIVATE covers both.
            hA, hB = 2 * hp, 2 * hp + 1
            qft, kft = hp, 4 + hp
            for ic in range(4):          # i-chunks of 512
                upsA = upp.tile([65, 512], f32, tag="upsA")
                upsB = upp.tile([65, 512], f32, tag="upsB")
                prev_pt = None
                for j in range(JT):
                    dt_ = dotsp.tile([P, 1024], f32, tag="dt")
                    nc.tensor.matmul(
                        dt_[:, 0:512],
                        qkT[0:64, kft, ds(j * P, P)],
                        qkT[0:64, qft, ds(ic * 512, 512)],
                        start=True, stop=True,
                    )
                    nc.tensor.matmul(
                        dt_[:, 512:1024],
                        qkT[64:128, kft, ds(j * P, P)],
                        qkT[64:128, qft, ds(ic * 512, 512)],
                        start=True, stop=True,
                    )
                    pt = ptp.tile([P, 1024], bf16, tag="pt")
                    nc.scalar.activation(pt[:], dt_[:], Exp, scale=0.125)
                    if prev_pt is not None:
                        jj = j - 1
                        nc.tensor.matmul(
                            upsA[:], vp_heads[:, jj, hA, :],
                            prev_pt[:, 0:512],
                            start=(jj == 0), stop=False,
                        )
                        nc.tensor.matmul(
                            upsB[:], vp_heads[:, jj, hB, :],
                            prev_pt[:, 512:1024],
                            start=(jj == 0), stop=False,
                        )
                    prev_pt = pt
                jj = JT - 1
                nc.tensor.matmul(
                    upsA[:], vp_heads[:, jj, hA, :], prev_pt[:, 0:512],
                    start=False, stop=True,
                )
                nc.tensor.matmul(
                    upsB[:], vp_heads[:, jj, hB, :], prev_pt[:, 512:1024],
                    start=False, stop=True,
                )
                emit_epilogue(upsA, ic, hA, tpp)
                emit_epilogue(upsB, ic, hB, tpp)

        # ---- emission ----
        # startup scope: x-transpose psums live only here (4 banks), so the
        # attention pools below get their banks back.
        with tc.tile_pool(name="tpsx", bufs=4, space="PSUM") as tpsx:
            emit_w()
            # x-tile batches interleaved with pair-0 QKV so the in-order PE
            # queue reaches the first dots as soon as w + 4 x-tiles are in.
            for batch in range(4):
                for it in range(4 * batch, 4 * batch + 4):
                    emit_x_tile(it, tpsx)
                emit_qk_group(0, batch)      # pair-0 q chunk
                emit_qk_group(4, batch)      # pair-0 k chunk
                for it in range(4 * batch, 4 * batch + 4):
                    emit_v_group(0, it)
        with tc.tile_pool(name="dotsp", bufs=2, space="PSUM") as dotsp, \
             tc.tile_pool(name="upp", bufs=1, space="PSUM") as upp, \
             tc.tile_pool(name="tpp", bufs=1, space="PSUM") as tpp:
            emit_attention_pair(0, dotsp, upp, tpp)
            for hp in range(1, 4):
                emit_qkv_for_pair(hp)
                emit_attention_pair(hp, dotsp, upp, tpp)

    nc.finalize()
    return nc


def _get_nc():
    if "nc" not in _CACHE:
        _CACHE["nc"] = _build()
    return _CACHE["nc"]


def kernel(x: np.ndarray, w_qkv: np.ndarray) -> np.ndarray:
    from concourse.bass_utils import run_bass_kernel_spmd

    x = np.asarray(x, dtype=np.float32)
    w_qkv = np.asarray(w_qkv, dtype=np.float32)

    in_maps = []
    for c in range(NCORES):
        b, hh = c // 2, c % 2
        qo = hh * FQ
        ws = np.concatenate(
            [w_qkv[:, qo:qo + FQ],
             w_qkv[:, INNER + qo:INNER + qo + FQ],
             w_qkv[:, 2 * INNER + qo:2 * INNER + qo + FQ]], axis=1)
        in_maps.append({
            "x": np.ascontiguousarray(x[b]),
            "w": np.ascontiguousarray(ws),
        })

    nc = _get_nc()
    res = run_bass_kernel_spmd(nc, in_maps, core_ids=list(range(NCORES)))

    out = np.empty((B, N, INNER), np.float32)
    for c in range(NCORES):
        b, hh = c // 2, c % 2
        out[b, :, hh * FQ:(hh + 1) * FQ] = res.results[c]["o"]
    return out
